# revision 24
# baseline (speedup 1.0000x reference)
"""CenterLoss Trainium2 kernel.

Computes, for features (B=16384, D=256), int targets (B,), centers (C=100000, D):
  loss        = mean((features - centers[targets])**2)
  new_centers = centers - delta
  delta[c]    = 0.5 * sum_{i: t_i==c}(centers[c] - f_i) / (count_c + 1)

Sharding: centers row-wise over 8 NeuronCores (12500 classes each). Batch rows
are routed on the host to the core owning their target class, sorted by class,
and packed into 128-row tiles such that no class straddles a tile boundary.
Rows are laid out partition-major: SBUF partition p, tile t holds batch row
t*128+p, so a single DMA loads all batch rows.

Per-core device program (SPMD — one Bass program, per-core data):
  - bulk copy of the centers shard into the output shard (absent classes),
    spread over both HWDGE rings (SP + ACT) to run chunks in parallel
  - per 128-row tile (17 tiles):
      indirect-DMA gather of the tile's target-center rows (one offset per
      partition — hardware reads only one index per partition per call),
      diff = c_t - f,
      selection matrix sel[i,j] = (idx[i]==idx[j]) via TensorE transpose + is_equal,
      segment sum  ssum = sel @ diff   (TensorE, PSUM),
      counts = row-sum(sel);  newrow = c_t - 0.5/(count+1) * ssum,
      indirect-DMA scatter of newrow into the output shard (after the bulk copy;
      duplicate rows of a class scatter identical values)
  - loss = sum(diff^2) reduced to (128,1) partials; host sums and divides

Padded rows carry class index CPC (=12500) which points at an appended
all-zero centers row and a scratch output row, so they contribute nothing.
"""

import numpy as np

NUM_CLASSES = 100000
FEAT_DIM = 256
BATCH = 16384
REG_LAMBDA = 1.0
REG_ALPHA = 0.5
NCORES = 8
CPC = NUM_CLASSES // NCORES  # classes per core
P = 128

_program_cache = {}

# dev knobs (test.py may set TRACE=True to profile; harness never touches these)
TRACE = False
LAST_RESULT = None


def _build_program(nt, iters=1, do_copies=True, do_rest=True, tile_ranges=None):
    """Build the SPMD Bass program for nt 128-row batch tiles per core.
    iters>1 wraps the whole body in an on-device For_i loop (used only by
    test.py to time the kernel; results are identical since the body is
    idempotent)."""
    import concourse.bass as bass
    import concourse.bacc as bacc
    import concourse.mybir as mybir
    import concourse.tile as tile
    from concourse.masks import make_identity

    nc = bacc.Bacc(None, target_bir_lowering=False)
    f32 = mybir.dt.float32
    D = FEAT_DIM
    centers = nc.dram_tensor("centers", [CPC + 1, D], f32, kind="ExternalInput")
    feats = nc.dram_tensor("feats", [P, nt * D], f32, kind="ExternalInput")
    lidx = nc.dram_tensor("lidx", [P, nt], mybir.dt.int32, kind="ExternalInput")
    outc = nc.dram_tensor("outc", [CPC + 1, D], f32, kind="ExternalOutput")
    lossv = nc.dram_tensor("lossv", [P, 1], f32, kind="ExternalOutput")

    with tile.TileContext(nc) as tc:
        with (
            tc.tile_pool(name="const", bufs=1) as cpool,
            tc.tile_pool(name="sbuf", bufs=3) as sbuf,
            tc.tile_pool(name="psum", bufs=3, space="PSUM") as psum,
        ):
            identity = cpool.tile([P, P], f32)
            make_identity(nc, identity[:])

            def _rest_copy_only(_iv=None):
                rows_total = CPC + 1
                sp_rows = (int(rows_total * 0.45) // 4) * 4
                for eng, lo, hi in ((nc.sync, 0, sp_rows), (nc.scalar, sp_rows, rows_total)):
                    n = 3
                    step = ((-(-(hi - lo) // n)) // 4 + 1) * 4
                    for a in range(lo, hi, step):
                        b = min(a + step, hi)
                        eng.dma_start(out=outc[a:b, :], in_=centers[a:b, :])
                nc.sync.dma_start(out=lossv[:, :], in_=identity[:, :1])

            def body(_iv=None):
                if not do_rest:
                    _rest_copy_only(_iv)
                    return
                # batch rows + indices first so gathers/compute start early
                lidx_sb = cpool.tile([P, nt], mybir.dt.int32, tag="lidx_sb")
                nc.sync.dma_start(out=lidx_sb[:], in_=lidx[:, :])
                f_all = cpool.tile([P, nt * D], f32, tag="f_all")
                nc.sync.dma_start(out=f_all[:], in_=feats[:, :])

                # bulk copy centers shard -> output shard (DRAM->DRAM).
                # HWDGE DMAs are FIFO per issuing engine, so spread chunks
                # over both HWDGE rings (SP + ACT) to run in parallel; the
                # gpsimd SWDGE ring is kept free for the indirect DMAs.
                # SP also carries the f_all load, so give it a smaller share.
                # Chunk boundaries stay multiples of 4 rows (4KB) — misaligned
                # DRAM offsets halve the modeled DMA rate.
                # Row CPC (scratch row for padded batch rows) is written only
                # by scatters, so it is excluded from the copy — padded rows
                # then never conflict with any copy chunk.
                rows_total = CPC
                sp_rows = (int(rows_total * 0.45) // 4) * 4
                copy_insts = []  # (row_lo, row_hi, inst)
                if do_copies:
                    for eng, lo, hi in ((nc.sync, 0, sp_rows), (nc.scalar, sp_rows, rows_total)):
                        n = 3
                        step = ((-(-(hi - lo) // n)) // 4 + 1) * 4
                        for a in range(lo, hi, step):
                            b = min(a + step, hi)
                            ins = eng.dma_start(out=outc[a:b, :], in_=centers[a:b, :])
                            copy_insts.append((a, b, ins))

                c_all = cpool.tile([P, nt * D], f32, tag="c_all")
                diff_all = cpool.tile([P, nt * D], f32, tag="diff_all")
                new_all = cpool.tile([P, nt * D], f32, tag="new_all")

                for t in range(nt):
                    sl = slice(t * D, (t + 1) * D)

                    # gather this tile's target-center rows (one index per
                    # partition — HW supports only (128,1) offset APs)
                    nc.gpsimd.indirect_dma_start(
                        out=c_all[:, sl],
                        out_offset=None,
                        in_=centers[:],
                        in_offset=bass.IndirectOffsetOnAxis(
                            ap=lidx_sb[:, t : t + 1], axis=0
                        ),
                    )

                    # selection matrix sel[i,j] = (idx[i] == idx[j])
                    lidxf = sbuf.tile([P, 1], f32, tag="lidxf")
                    nc.vector.tensor_copy(out=lidxf[:], in_=lidx_sb[:, t : t + 1])
                    lidxT_ps = psum.tile([P, P], f32, space="PSUM", tag="lidxT_ps")
                    nc.tensor.transpose(
                        out=lidxT_ps[:],
                        in_=lidxf[:].to_broadcast([P, P]),
                        identity=identity[:],
                    )
                    lidxT = sbuf.tile([P, P], f32, tag="lidxT")
                    nc.vector.tensor_copy(out=lidxT[:], in_=lidxT_ps[:])
                    sel = sbuf.tile([P, P], f32, tag="sel")
                    nc.vector.tensor_tensor(
                        out=sel[:],
                        in0=lidxf[:].to_broadcast([P, P])[:],
                        in1=lidxT[:],
                        op=mybir.AluOpType.is_equal,
                    )

                    # g = REG_ALPHA / (count + 1), per partition
                    counts = sbuf.tile([P, 1], f32, tag="counts")
                    nc.vector.reduce_sum(
                        out=counts[:], in_=sel[:], axis=mybir.AxisListType.X
                    )
                    cp1 = sbuf.tile([P, 1], f32, tag="cp1")
                    nc.vector.tensor_scalar_add(
                        out=cp1[:], in0=counts[:], scalar1=1.0
                    )
                    ginv = sbuf.tile([P, 1], f32, tag="ginv")
                    nc.vector.reciprocal(out=ginv[:], in_=cp1[:])
                    g = sbuf.tile([P, 1], f32, tag="g")
                    nc.vector.tensor_scalar_mul(
                        out=g[:], in0=ginv[:], scalar1=REG_ALPHA
                    )

                    # diff = c_t - f ; ssum = sel @ diff (per-class sums)
                    nc.vector.tensor_tensor(
                        out=diff_all[:, sl],
                        in0=c_all[:, sl],
                        in1=f_all[:, sl],
                        op=mybir.AluOpType.subtract,
                    )
                    ssum = psum.tile([P, D], f32, space="PSUM", tag="ssum")
                    nc.tensor.matmul(
                        out=ssum[:],
                        lhsT=sel[:],
                        rhs=diff_all[:, sl],
                        start=True,
                        stop=True,
                    )

                    # newrow = c_t - g * ssum
                    delta = sbuf.tile([P, D], f32, tag="delta")
                    nc.vector.tensor_scalar(
                        out=delta[:],
                        in0=ssum[:],
                        scalar1=g[:, :1],
                        scalar2=None,
                        op0=mybir.AluOpType.mult,
                    )
                    nc.vector.tensor_tensor(
                        out=new_all[:, sl],
                        in0=c_all[:, sl],
                        in1=delta[:],
                        op=mybir.AluOpType.subtract,
                    )

                    # scatter updated rows; must land after the bulk copy of
                    # the class range this tile touches. Classes are sorted,
                    # so each tile overlaps only 1-2 copy chunks — its scatter
                    # fires as soon as those chunks land, overlapping the rest
                    # of the copy instead of serializing behind all of it.
                    scat = nc.gpsimd.indirect_dma_start(
                        out=outc[:],
                        out_offset=bass.IndirectOffsetOnAxis(
                            ap=lidx_sb[:, t : t + 1], axis=0
                        ),
                        in_=new_all[:, sl],
                        in_offset=None,
                    )
                    if tile_ranges is not None:
                        t_lo, t_hi = tile_ranges[t]
                    else:
                        t_lo, t_hi = 0, CPC
                    for c_lo, c_hi, ci in copy_insts:
                        if c_lo < t_hi + 1 and t_lo < c_hi:
                            tile.add_dep_helper(
                                getattr(scat, "ins", scat),
                                getattr(ci, "ins", ci),
                                reason="scatter after bulk centers copy",
                            )

                # loss partials: acc[p] = sum_t sum_d diff^2
                d2 = cpool.tile([P, nt * D], f32, tag="d2")
                nc.vector.tensor_tensor(
                    out=d2[:],
                    in0=diff_all[:],
                    in1=diff_all[:],
                    op=mybir.AluOpType.mult,
                )
                acc = sbuf.tile([P, 1], f32, tag="acc")
                nc.vector.reduce_sum(
                    out=acc[:], in_=d2[:], axis=mybir.AxisListType.X
                )
                nc.sync.dma_start(out=lossv[:, :], in_=acc[:])

            if iters == 1:
                body()
            else:
                with tc.For_i(0, iters, 1) as _i:
                    body(_i)

    nc.compile()
    return nc


def _shard_inputs(features, targets, centers):
    """Route batch rows to the core owning their target class; sort by class
    and pack into 128-row tiles so no class straddles a tile boundary.
    Layout is partition-major: partition p, tile t <- batch row t*128+p."""
    t = np.asarray(targets).astype(np.int64)
    order = np.argsort(t, kind="stable")
    ts = t[order]
    features = np.asarray(features, np.float32)
    centers = np.asarray(centers, np.float32)

    per_core = []
    nt_max = 1
    for k in range(NCORES):
        lo, hi = np.searchsorted(ts, [k * CPC, (k + 1) * CPC])
        rows = order[lo:hi]
        loc = (ts[lo:hi] - k * CPC).astype(np.int64)
        n = rows.shape[0]
        # run boundaries of equal class
        if n:
            starts = np.flatnonzero(np.r_[True, loc[1:] != loc[:-1]])
            lens = np.diff(np.r_[starts, n])
        else:
            starts = np.empty(0, np.int64)
            lens = np.empty(0, np.int64)
        pos = np.empty(n, np.int64)
        tile_i = 0
        fill = 0
        for s, l in zip(starts, lens):
            l = int(l)
            if l > P:
                raise ValueError("class with more than 128 batch rows")
            if fill + l > P:
                tile_i += 1
                fill = 0
            pos[s : s + l] = tile_i * P + fill + np.arange(l)
            fill += l
            if fill == P:
                tile_i += 1
                fill = 0
        nt_k = tile_i + (1 if fill > 0 else 0)
        nt_k = max(nt_k, 1)
        nt_max = max(nt_max, nt_k)
        per_core.append((k, rows, loc, pos, nt_k))

    in_maps = []
    for k, rows, loc, pos, _ in per_core:
        feats_lin = np.zeros((nt_max * P, FEAT_DIM), np.float32)
        lidx_lin = np.full(nt_max * P, CPC, np.int32)
        if rows.shape[0]:
            feats_lin[pos] = features[rows]
            lidx_lin[pos] = loc.astype(np.int32)
        # partition-major: [p, t*D:(t+1)*D] = row t*128+p
        feats_k = np.ascontiguousarray(
            feats_lin.reshape(nt_max, P, FEAT_DIM)
            .transpose(1, 0, 2)
            .reshape(P, nt_max * FEAT_DIM)
        )
        lidx_k = np.ascontiguousarray(lidx_lin.reshape(nt_max, P).T)
        centers_k = np.zeros((CPC + 1, FEAT_DIM), np.float32)
        centers_k[:CPC] = centers[k * CPC : (k + 1) * CPC]
        in_maps.append({"centers": centers_k, "feats": feats_k, "lidx": lidx_k})
    return in_maps, nt_max


def _tile_ranges(in_maps, nt):
    """Per-tile real-class range (union over cores) for scatter/copy deps."""
    tile_ranges = []
    for t in range(nt):
        lo, hi = CPC, -1
        for m in in_maps:
            col = m["lidx"][:, t]
            real = col[col < CPC]
            if real.size:
                lo = min(lo, int(real.min()))
                hi = max(hi, int(real.max()))
        tile_ranges.append((lo, hi))
    return tuple(tile_ranges)


def kernel(features, targets, centers):
    from concourse.bass_utils import run_bass_kernel_spmd

    in_maps, nt = _shard_inputs(features, targets, centers)
    tile_ranges = _tile_ranges(in_maps, nt)

    key = (nt, tile_ranges)
    nc = _program_cache.get(key)
    if nc is None:
        nc = _build_program(nt, tile_ranges=tile_ranges)
        _program_cache[key] = nc

    res = run_bass_kernel_spmd(
        nc, in_maps, core_ids=list(range(NCORES)), trace=TRACE
    )
    global LAST_RESULT
    LAST_RESULT = res

    new_centers = np.concatenate(
        [res.results[k]["outc"][:CPC] for k in range(NCORES)], axis=0
    )
    loss_sum = sum(
        float(res.results[k]["lossv"].astype(np.float64).sum()) for k in range(NCORES)
    )
    loss = np.float32(loss_sum / (BATCH * FEAT_DIM) * REG_LAMBDA)
    return loss, new_centers


# revision 37
# speedup vs baseline: 1.0070x; 1.0070x over previous
"""CenterLoss Trainium2 kernel.

Computes, for features (B=16384, D=256), int targets (B,), centers (C=100000, D):
  loss        = mean((features - centers[targets])**2)
  new_centers = centers - delta
  delta[c]    = 0.5 * sum_{i: t_i==c}(centers[c] - f_i) / (count_c + 1)

Sharding: centers row-wise over 8 NeuronCores (12500 classes each). Batch rows
are routed on the host to the core owning their target class, sorted by class,
and packed into 128-row tiles such that no class straddles a tile boundary.
Rows are laid out partition-major: SBUF partition p, tile t holds batch row
t*128+p, so a single DMA loads all batch rows.

Per-core device program (SPMD — one Bass program, per-core data):
  - bulk copy of the centers shard into the output shard (absent classes),
    spread over both HWDGE rings (SP + ACT) to run chunks in parallel; the
    scratch row CPC is excluded so padded rows never conflict with the copy
  - 17 indirect-DMA gathers of target-center rows issued up-front (no copy
    dependencies, so the serialized SWDGE ring streams them back-to-back)
  - per 128-row tile (17 tiles):
      diff = c_t - f,
      selection matrix sel[i,j] = (idx[i]==idx[j]) via TensorE transpose + is_equal,
      segment sum  ssum = sel @ diff   (TensorE, PSUM),
      counts = row-sum(sel);  newrow = c_t - 0.5/(count+1) * ssum,
  - 17 indirect-DMA scatters of the new rows, issued after all compute and
    ordered by when their copy chunks complete. Classes are sorted, so each
    tile's scatter depends only on the 1-2 copy chunks covering its class
    range; the completion ordering avoids head-of-line blocking on the
    serialized SWDGE ring. Duplicate rows of a class scatter identical values.
  - loss = sum(diff^2) reduced to (128,1) partials; host sums and divides

Padded rows carry class index CPC (=12500) which points at an appended
all-zero centers row and a scratch output row, so they contribute nothing;
the host drops row CPC when unsharding.
"""

import numpy as np

NUM_CLASSES = 100000
FEAT_DIM = 256
BATCH = 16384
REG_LAMBDA = 1.0
REG_ALPHA = 0.5
NCORES = 8
CPC = NUM_CLASSES // NCORES  # classes per core
P = 128

_program_cache = {}

# dev knobs (test.py may set TRACE=True to profile; harness never touches these)
TRACE = False
LAST_RESULT = None


def _build_program(nt, iters=1, tile_ranges=None):
    """Build the SPMD Bass program for nt 128-row batch tiles per core.
    iters>1 wraps the whole body in an on-device For_i loop (used only by
    test.py to time the kernel; results are identical since the body is
    idempotent)."""
    import concourse.bass as bass
    import concourse.bacc as bacc
    import concourse.mybir as mybir
    import concourse.tile as tile
    from concourse.masks import make_identity

    nc = bacc.Bacc(None, target_bir_lowering=False)
    f32 = mybir.dt.float32
    i16 = mybir.dt.int16
    D = FEAT_DIM
    NI = nt * P
    centers = nc.dram_tensor("centers", [CPC + 1, D], f32, kind="ExternalInput")
    feats = nc.dram_tensor("feats", [P, nt * D], f32, kind="ExternalInput")
    lidx = nc.dram_tensor("lidx", [P, nt], mybir.dt.int32, kind="ExternalInput")
    outc = nc.dram_tensor("outc", [CPC + 1, D], f32, kind="ExternalOutput")
    lossv = nc.dram_tensor("lossv", [P, 1], f32, kind="ExternalOutput")

    with tile.TileContext(nc) as tc:
        with (
            tc.tile_pool(name="const", bufs=1) as cpool,
            tc.tile_pool(name="sbuf", bufs=3) as sbuf,
            tc.tile_pool(name="psum", bufs=3, space="PSUM") as psum,
        ):
            identity = cpool.tile([P, P], f32)
            make_identity(nc, identity[:])

            def body(_iv=None):
                # batch rows + indices first so the gather/compute start early
                lidx_sb = cpool.tile([P, nt], mybir.dt.int32, tag="lidx_sb")
                nc.sync.dma_start(out=lidx_sb[:], in_=lidx[:, :])
                f_all = cpool.tile([P, nt * D], f32, tag="f_all")
                nc.sync.dma_start(out=f_all[:], in_=feats[:, :])

                # bulk copy centers shard -> output shard (DRAM->DRAM) over
                # both HWDGE rings; SP also carries f_all so it gets less.
                # Chunk boundaries stay multiples of 4 rows (4KB alignment).
                rows_total = CPC  # scratch row CPC excluded (scatter-only)
                sp_rows = (int(rows_total * 0.45) // 4) * 4
                copy_insts = []  # (row_lo, row_hi, inst)
                for eng, lo, hi in (
                    (nc.sync, 0, sp_rows),
                    (nc.scalar, sp_rows, rows_total),
                ):
                    n = 3
                    step = ((-(-(hi - lo) // n)) // 4 + 1) * 4
                    for a in range(lo, hi, step):
                        b = min(a + step, hi)
                        ins = eng.dma_start(out=outc[a:b, :], in_=centers[a:b, :])
                        copy_insts.append((a, b, ins))

                c_all = cpool.tile([P, nt * D], f32, tag="c_all")
                diff_all = cpool.tile([P, nt * D], f32, tag="diff_all")
                new_all = cpool.tile([P, nt * D], f32, tag="new_all")

                # all gathers up-front: they carry no copy dependencies, so
                # the serialized SWDGE ring streams them back-to-back instead
                # of stalling behind scatter waits (head-of-line blocking)
                for t in range(nt):
                    nc.gpsimd.indirect_dma_start(
                        out=c_all[:, t * D : (t + 1) * D],
                        out_offset=None,
                        in_=centers[:],
                        in_offset=bass.IndirectOffsetOnAxis(
                            ap=lidx_sb[:, t : t + 1], axis=0
                        ),
                    )

                for t in range(nt):
                    sl = slice(t * D, (t + 1) * D)

                    # selection matrix sel[i,j] = (idx[i] == idx[j])
                    lidxf = sbuf.tile([P, 1], f32, tag="lidxf")
                    nc.vector.tensor_copy(out=lidxf[:], in_=lidx_sb[:, t : t + 1])
                    lidxT_ps = psum.tile([P, P], f32, space="PSUM", tag="lidxT_ps")
                    nc.tensor.transpose(
                        out=lidxT_ps[:],
                        in_=lidxf[:].to_broadcast([P, P]),
                        identity=identity[:],
                    )
                    lidxT = sbuf.tile([P, P], f32, tag="lidxT")
                    nc.vector.tensor_copy(out=lidxT[:], in_=lidxT_ps[:])
                    sel = sbuf.tile([P, P], f32, tag="sel")
                    nc.vector.tensor_tensor(
                        out=sel[:],
                        in0=lidxf[:].to_broadcast([P, P])[:],
                        in1=lidxT[:],
                        op=mybir.AluOpType.is_equal,
                    )

                    # g = REG_ALPHA / (count + 1), per partition
                    counts = sbuf.tile([P, 1], f32, tag="counts")
                    nc.vector.reduce_sum(
                        out=counts[:], in_=sel[:], axis=mybir.AxisListType.X
                    )
                    cp1 = sbuf.tile([P, 1], f32, tag="cp1")
                    nc.vector.tensor_scalar_add(out=cp1[:], in0=counts[:], scalar1=1.0)
                    ginv = sbuf.tile([P, 1], f32, tag="ginv")
                    nc.vector.reciprocal(out=ginv[:], in_=cp1[:])
                    g = sbuf.tile([P, 1], f32, tag="g")
                    nc.vector.tensor_scalar_mul(out=g[:], in0=ginv[:], scalar1=REG_ALPHA)

                    # diff = c_t - f ; ssum = sel @ diff (per-class sums)
                    nc.vector.tensor_tensor(
                        out=diff_all[:, sl],
                        in0=c_all[:, sl],
                        in1=f_all[:, sl],
                        op=mybir.AluOpType.subtract,
                    )
                    ssum = psum.tile([P, D], f32, space="PSUM", tag="ssum")
                    nc.tensor.matmul(
                        out=ssum[:],
                        lhsT=sel[:],
                        rhs=diff_all[:, sl],
                        start=True,
                        stop=True,
                    )

                    # newrow = c_t - g * ssum
                    delta = sbuf.tile([P, D], f32, tag="delta")
                    nc.vector.tensor_scalar(
                        out=delta[:],
                        in0=ssum[:],
                        scalar1=g[:, :1],
                        scalar2=None,
                        op0=mybir.AluOpType.mult,
                    )
                    nc.vector.tensor_tensor(
                        out=new_all[:, sl],
                        in0=c_all[:, sl],
                        in1=delta[:],
                        op=mybir.AluOpType.subtract,
                    )

                # scatters after all compute, ordered by when their copy
                # chunks complete. copy_insts[i] is the (i%3+1)-th chunk on
                # its ring (3 per ring, rings run concurrently), so sorting
                # tiles by chunk-position-within-ring lets early scatters
                # proceed while late copy chunks are still in flight.
                def dep_chunks(t):
                    if tile_ranges is not None:
                        t_lo, t_hi = tile_ranges[t]
                    else:
                        t_lo, t_hi = 0, CPC
                    return [
                        i
                        for i, (c_lo, c_hi, _ci) in enumerate(copy_insts)
                        if c_lo < t_hi + 1 and t_lo < c_hi
                    ]

                order = sorted(range(nt), key=lambda t: max(dep_chunks(t), default=-1) % 3)
                for t in order:
                    scat = nc.gpsimd.indirect_dma_start(
                        out=outc[:],
                        out_offset=bass.IndirectOffsetOnAxis(
                            ap=lidx_sb[:, t : t + 1], axis=0
                        ),
                        in_=new_all[:, t * D : (t + 1) * D],
                        in_offset=None,
                    )
                    for i in dep_chunks(t):
                        tile.add_dep_helper(
                            getattr(scat, "ins", scat),
                            getattr(copy_insts[i][2], "ins", copy_insts[i][2]),
                            reason="scatter after bulk centers copy",
                        )

                # loss partials: acc[p] = sum_t sum_d diff^2
                d2 = cpool.tile([P, nt * D], f32, tag="d2")
                nc.vector.tensor_tensor(
                    out=d2[:], in0=diff_all[:], in1=diff_all[:], op=mybir.AluOpType.mult
                )
                acc = sbuf.tile([P, 1], f32, tag="acc")
                nc.vector.reduce_sum(out=acc[:], in_=d2[:], axis=mybir.AxisListType.X)
                nc.sync.dma_start(out=lossv[:, :], in_=acc[:])

            if iters == 1:
                body()
            else:
                with tc.For_i(0, iters, 1) as _i:
                    body(_i)

    nc.compile()
    return nc


def _shard_inputs(features, targets, centers):
    """Route batch rows to the core owning their target class; sort by class
    and pack into 128-row tiles so no class straddles a tile boundary.
    Layout is partition-major: partition p, tile t <- batch row t*128+p."""
    t = np.asarray(targets).astype(np.int64)
    order = np.argsort(t, kind="stable")
    ts = t[order]
    features = np.asarray(features, np.float32)
    centers = np.asarray(centers, np.float32)

    per_core = []
    nt_max = 1
    for k in range(NCORES):
        lo, hi = np.searchsorted(ts, [k * CPC, (k + 1) * CPC])
        rows = order[lo:hi]
        loc = (ts[lo:hi] - k * CPC).astype(np.int64)
        n = rows.shape[0]
        if n:
            starts = np.flatnonzero(np.r_[True, loc[1:] != loc[:-1]])
            lens = np.diff(np.r_[starts, n])
        else:
            starts = np.empty(0, np.int64)
            lens = np.empty(0, np.int64)
        pos = np.empty(n, np.int64)
        tile_i = 0
        fill = 0
        for s, l in zip(starts, lens):
            l = int(l)
            if l > P:
                raise ValueError("class with more than 128 batch rows")
            if fill + l > P:
                tile_i += 1
                fill = 0
            pos[s : s + l] = tile_i * P + fill + np.arange(l)
            fill += l
            if fill == P:
                tile_i += 1
                fill = 0
        nt_k = tile_i + (1 if fill > 0 else 0)
        nt_k = max(nt_k, 1)
        nt_max = max(nt_max, nt_k)
        per_core.append((k, rows, loc, pos, nt_k))

    in_maps = []
    for k, rows, loc, pos, _ in per_core:
        feats_lin = np.zeros((nt_max * P, FEAT_DIM), np.float32)
        lidx_lin = np.full(nt_max * P, CPC, np.int32)
        if rows.shape[0]:
            feats_lin[pos] = features[rows]
            lidx_lin[pos] = loc.astype(np.int32)
        feats_k = np.ascontiguousarray(
            feats_lin.reshape(nt_max, P, FEAT_DIM)
            .transpose(1, 0, 2)
            .reshape(P, nt_max * FEAT_DIM)
        )
        lidx_k = np.ascontiguousarray(lidx_lin.reshape(nt_max, P).T)
        centers_k = np.zeros((CPC + 1, FEAT_DIM), np.float32)
        centers_k[:CPC] = centers[k * CPC : (k + 1) * CPC]
        in_maps.append({"centers": centers_k, "feats": feats_k, "lidx": lidx_k})
    return in_maps, nt_max


def _tile_ranges(in_maps, nt):
    """Per-tile real-class range (union over cores) for scatter/copy deps."""
    tile_ranges = []
    for t in range(nt):
        lo, hi = CPC, -1
        for m in in_maps:
            col = m["lidx"][:, t]
            real = col[col < CPC]
            if real.size:
                lo = min(lo, int(real.min()))
                hi = max(hi, int(real.max()))
        tile_ranges.append((lo, hi))
    return tuple(tile_ranges)


def kernel(features, targets, centers):
    from concourse.bass_utils import run_bass_kernel_spmd

    in_maps, nt = _shard_inputs(features, targets, centers)
    tile_ranges = _tile_ranges(in_maps, nt)

    key = (nt, tile_ranges)
    nc = _program_cache.get(key)
    if nc is None:
        nc = _build_program(nt, tile_ranges=tile_ranges)
        _program_cache[key] = nc

    res = run_bass_kernel_spmd(nc, in_maps, core_ids=list(range(NCORES)), trace=TRACE)
    global LAST_RESULT
    LAST_RESULT = res

    new_centers = np.concatenate(
        [res.results[k]["outc"][:CPC] for k in range(NCORES)], axis=0
    )
    loss_sum = sum(
        float(res.results[k]["lossv"].astype(np.float64).sum()) for k in range(NCORES)
    )
    loss = np.float32(loss_sum / (BATCH * FEAT_DIM) * REG_LAMBDA)
    return loss, new_centers
